# revision 1
# baseline (speedup 1.0000x reference)
"""Trainium2 Bass kernel for nn_MultiHeadAttention_64106681860559.

Fused single-score-matrix MHA: qkv = x@Wqkv+b; S = q k^T/8; attn = softmax(S);
out = (attn @ v) @ Wout + bout.   x:[4096,1024] fp32 -> y:[4096,1024] fp32.

Strategy: shard tokens (dim 0) across 8 cores (512 tokens each).
 - Each core computes kT/qT (feature-major) and v (token-major) projections
   for its shard from full weights.
 - AllGather #1 shares kT (full K^T needed by every core's scores);
   AllGather #2 shares v.
 - Scores are computed transposed (S^T = K @ q_i^T -> [4096 keys, 512 q]) so
   no on-chip transposes are needed anywhere; softmax runs over the partition
   (key) axis: exp on ACT with a global shift (softmax is shift-invariant),
   denominator via ones-vector matmul accumulated in PSUM, broadcast of the
   reciprocal via a tiny fp32 matmul.
 - U = V^T E accumulated over key chunks (PSUM for groups of 8, summed in
   SBUF fp32), normalized, then yT = Wout^T outT + bout.
All matmul operands are fp16 (1 cycle/row on the PE, halves DMA/SBUF);
accumulation is fp32 in PSUM; bias adds/normalization in fp32.
Measured end-to-end error vs fp32 reference: ~2e-3 scale-relative absmax.
"""
import sys
import numpy as np

for _p in ("/opt/trn_rl_repo", "/root/.axon_site/_ro/trn_rl_repo"):
    if _p not in sys.path:
        sys.path.insert(0, _p)

import concourse.bass as bass  # noqa: E402
import concourse.tile as tile  # noqa: E402
from concourse import bacc, mybir  # noqa: E402
from concourse.bass_utils import run_bass_kernel_spmd  # noqa: E402

R = 8            # cores
N = 4096         # tokens
S = N // R       # 512 tokens per shard
L = 1024         # latent
F = 1024         # q/k/v feature width (H*Dk = H*Dv)
KO = L // 128    # 8 latent chunks
FO = F // 128    # 8 feature chunks
MT = S // 128    # 4 token tiles per shard
NKC = N // 128   # 32 key chunks
EXP_SHIFT = -16.0   # global shift inside exp; softmax-invariant, keeps E in fp16 range
SCALE = 0.125       # 1/sqrt(Dk)

f16 = mybir.dt.float16
f32 = mybir.dt.float32

_cached = None


def _build():
    nc = bacc.Bacc("TRN2", target_bir_lowering=False, debug=False, num_devices=R)

    xT = nc.dram_tensor("xT", [L, S], f16, kind="ExternalInput")
    wq = nc.dram_tensor("wq", [L, F], f16, kind="ExternalInput")
    wk = nc.dram_tensor("wk", [L, F], f16, kind="ExternalInput")
    wv = nc.dram_tensor("wv", [L, F], f16, kind="ExternalInput")
    wo = nc.dram_tensor("wo", [F, L], f16, kind="ExternalInput")
    bq = nc.dram_tensor("bq", [128, FO], f32, kind="ExternalInput")
    bk = nc.dram_tensor("bk", [128, FO], f32, kind="ExternalInput")
    bvb = nc.dram_tensor("bvb", [128, F], f32, kind="ExternalInput")
    bo = nc.dram_tensor("bo", [128, KO], f32, kind="ExternalInput")
    yT = nc.dram_tensor("yT", [L, S], f32, kind="ExternalOutput")

    with tile.TileContext(nc) as tc:
        with tc.tile_pool(name="const", bufs=1) as const, \
             tc.tile_pool(name="dram", bufs=1, space="DRAM") as dram, \
             tc.tile_pool(name="wpool", bufs=2) as wpool, \
             tc.tile_pool(name="ktpool", bufs=3) as ktpool, \
             tc.tile_pool(name="vpool", bufs=8) as vpool, \
             tc.tile_pool(name="epool", bufs=NKC) as epool, \
             tc.tile_pool(name="ypool", bufs=3) as ypool, \
             tc.tile_pool(name="ps_proj", bufs=4, space="PSUM") as ps_proj, \
             tc.tile_pool(name="ps_u", bufs=2, space="PSUM") as ps_upool, \
             tc.tile_pool(name="ps_d", bufs=1, space="PSUM") as ps_dpool, \
             tc.tile_pool(name="ps_one", bufs=1, space="PSUM") as ps_one:

            # ---- inputs needed first: x and Wk (everything else deferred) ----
            xT16 = const.tile([128, KO, S], f16, name="xT16")
            nc.sync.dma_start(xT16[:], xT.ap().rearrange("(ko p) t -> p ko t", p=128))
            bk_s = const.tile([128, FO], f32, name="bk_s")
            nc.sync.dma_start(bk_s[:], bk.ap())
            wk16 = wpool.tile([128, KO, F], f16, tag="w", name="wk16")
            wk_view = wk.ap().rearrange("(ko p) f -> p ko f", p=128)
            # two halves so the first k-chains start before the full load
            nc.sync.dma_start(wk16[:, :, :512], wk_view[:, :, :512])
            nc.sync.dma_start(wk16[:, :, 512:], wk_view[:, :, 512:])

            ones_c32 = const.tile([128, 1], f32, name="ones_c32")
            nc.vector.memset(ones_c32[:], 1.0)
            ones_r32 = const.tile([1, 128], f32, name="ones_r32")
            nc.vector.memset(ones_r32[:], 1.0)
            expb = const.tile([128, 1], f32, name="expb")
            nc.vector.memset(expb[:], EXP_SHIFT)

            qT16 = const.tile([128, FO, S], f16, name="qT16")
            kT16 = const.tile([128, FO, S], f16, name="kT16")
            v16 = const.tile([128, MT, F], f16, name="v16")
            u_sb = const.tile([128, FO, S], f32, name="u_sb")
            outT16 = const.tile([128, FO, S], f16, name="outT16")
            rb32 = const.tile([128, S], f32, name="rb32")
            recip32 = const.tile([1, S], f32, name="recip32")

            # ---- collective buffers (K and V each gathered in two halves
            #      so downstream compute starts before the full gather) ----
            SH = S // 2  # 256 tokens per k-half
            k_in = [dram.tile([128, FO * SH], f16, name=f"k_in{h}")
                    for h in range(2)]
            k_gath = [dram.tile([R, 128, FO * SH], f16, name=f"k_gath{h}",
                                addr_space="Shared") for h in range(2)]
            v_in = [dram.tile([128, 2 * F], f16, name=f"v_in{h}")
                    for h in range(2)]
            v_gath = [dram.tile([R, 128, 2 * F], f16, name=f"v_gath{h}",
                                addr_space="Shared") for h in range(2)]
            # ---- phase A: projections (k -> AG1, v -> AG2, then q) ----
            def proj_feature_major(w_dram, bias_s, out16):
                # out[feat, tok] = W^T @ x^T ; lhsT = W[latent, feat]
                w16 = wpool.tile([128, KO, F], f16, tag="w", name="w16")
                nc.sync.dma_start(
                    w16[:], w_dram.ap().rearrange("(ko p) f -> p ko f", p=128))
                for fo in range(FO):
                    ps = ps_proj.tile([128, S], f32, tag="ps_proj", name="ps_p")
                    for ko in range(KO):
                        nc.tensor.matmul(
                            ps[:], w16[:, ko, fo * 128:(fo + 1) * 128],
                            xT16[:, ko, :],
                            start=(ko == 0), stop=(ko == KO - 1))
                    nc.vector.tensor_scalar_add(
                        out=out16[:, fo, :], in0=ps[:],
                        scalar1=bias_s[:, fo:fo + 1])

            def ag_k(h):
                nc.sync.dma_start(
                    k_in[h].rearrange("p (fo t) -> p fo t", t=SH),
                    kT16[:, :, h * SH:(h + 1) * SH])
                nc.gpsimd.collective_compute(
                    "AllGather", mybir.AluOpType.bypass,
                    replica_groups=[list(range(R))],
                    ins=[k_in[h][:].opt()], outs=[k_gath[h][:].opt()])

            # k projection; gathers go out as token-halves, interleaved with
            # the v-halves below (queue order k1, v1, k2, v2 matches the
            # ST/U phase interleave)
            for fo in range(FO):
                ps = ps_proj.tile([128, S], f32, tag="ps_proj", name="ps_k")
                for ko in range(KO):
                    nc.tensor.matmul(
                        ps[:], wk16[:, ko, fo * 128:(fo + 1) * 128],
                        xT16[:, ko, :],
                        start=(ko == 0), stop=(ko == KO - 1))
                nc.vector.tensor_scalar_add(
                    out=kT16[:, fo, :], in0=ps[:],
                    scalar1=bk_s[:, fo:fo + 1])
            ag_k(0)

            # q projection fills the AllGather wait
            bq_s = const.tile([128, FO], f32, name="bq_s")
            nc.sync.dma_start(bq_s[:], bq.ap())
            proj_feature_major(wq, bq_s, qT16)

            # deferred constant load (needed for v projection)
            bvb_s = const.tile([128, F], f32, name="bvb_s")
            nc.sync.dma_start(bvb_s[:], bvb.ap())

            # v[tok, feat] = x @ Wv ; lhsT = xT chunk [latent, tok]
            wv16 = wpool.tile([128, KO, F], f16, tag="w", name="wv16")
            nc.sync.dma_start(
                wv16[:], wv.ap().rearrange("(ko p) f -> p ko f", p=128))
            for mt in range(MT):
                for nh in range(2):
                    ps = ps_proj.tile([128, 512], f32, tag="ps_proj", name="ps_v")
                    for ko in range(KO):
                        nc.tensor.matmul(
                            ps[:], xT16[:, ko, mt * 128:(mt + 1) * 128],
                            wv16[:, ko, nh * 512:(nh + 1) * 512],
                            start=(ko == 0), stop=(ko == KO - 1))
                    nc.vector.tensor_tensor(
                        v16[:, mt, nh * 512:(nh + 1) * 512], ps[:],
                        bvb_s[:, nh * 512:(nh + 1) * 512], mybir.AluOpType.add)
                if mt == 1:
                    # v half 0 right after its tiles; then the second k half
                    nc.sync.dma_start(
                        v_in[0].rearrange("p (mt f) -> p mt f", f=F),
                        v16[:, 0:2, :])
                    nc.gpsimd.collective_compute(
                        "AllGather", mybir.AluOpType.bypass,
                        replica_groups=[list(range(R))],
                        ins=[v_in[0][:].opt()], outs=[v_gath[0][:].opt()])
                    ag_k(1)
                elif mt == 3:
                    nc.sync.dma_start(
                        v_in[1].rearrange("p (mt f) -> p mt f", f=F),
                        v16[:, 2:4, :])
                    nc.gpsimd.collective_compute(
                        "AllGather", mybir.AluOpType.bypass,
                        replica_groups=[list(range(R))],
                        ins=[v_in[1][:].opt()], outs=[v_gath[1][:].opt()])

            # deferred loads for the output projection
            bo_s = const.tile([128, KO], f32, name="bo_s")
            nc.sync.dma_start(bo_s[:], bo.ap())
            wo16 = const.tile([128, FO, L], f16, name="wo16")
            nc.sync.dma_start(wo16[:], wo.ap().rearrange("(fo p) l -> p fo l", p=128))

            # rank rotation: rot[j] = (pid + 1 + j) % 8, j = 0..6 covers all
            # peers; slot j of the main phases holds peer rot[j] on every core.
            rot_sv = []
            pid = nc.sync.partition_id()
            pid_r = nc.sync.to_reg(pid)
            for j in range(R - 1):
                t = nc.sync.alloc_register(f"rot_{j}")
                nc.sync.reg_add(t, pid_r, 1 + j)
                nc.sync.reg_mod(t, t, R)
                rot_sv.append(
                    nc.sync.snap(t, donate=True, min_val=0, max_val=R - 1))

            t_sum = const.tile([128, S], f32, name="t_sum")
            e_tiles = {}
            first_e = True

            def st_chain(kt_ap, e_key):
                """One scores^T chain + exp + denominator partial."""
                nonlocal first_e
                ps_s = ps_proj.tile([128, S], f32, tag="ps_proj", name="ps_s")
                for fo in range(FO):
                    nc.tensor.matmul(
                        ps_s[:], kt_ap[:, fo, :], qT16[:, fo, :],
                        start=(fo == 0), stop=(fo == FO - 1))
                e_t = epool.tile([128, S], f16, tag="e", name="e_t")
                nc.scalar.activation(
                    e_t[:], ps_s[:], mybir.ActivationFunctionType.Exp,
                    bias=expb[:], scale=SCALE)
                if first_e:
                    nc.vector.tensor_copy(out=t_sum[:], in_=e_t[:])
                    first_e = False
                else:
                    nc.vector.tensor_tensor(
                        t_sum[:], t_sum[:], e_t[:], mybir.AluOpType.add)
                e_tiles[e_key] = e_t

            first_u = [True] * FO

            def u_pass(chunk_list, v_ap_fn):
                """Accumulate u_sb[fo2] += sum over chunk_list of V^T E."""
                for fo2 in range(FO):
                    ps_u = ps_upool.tile([128, S], f32, tag="ps_u", name="ps_u")
                    n = len(chunk_list)
                    for j, ck in enumerate(chunk_list):
                        nc.tensor.matmul(
                            ps_u[:], v_ap_fn(ck, fo2), e_tiles[ck][:],
                            start=(j == 0), stop=(j == n - 1))
                    if first_u[fo2]:
                        nc.vector.tensor_copy(out=u_sb[:, fo2, :], in_=ps_u[:])
                        first_u[fo2] = False
                    else:
                        nc.vector.tensor_tensor(
                            u_sb[:, fo2, :], u_sb[:, fo2, :], ps_u[:],
                            mybir.AluOpType.add)

            # ---- prologue: own-rank scores + U from local SBUF, runs while
            #      the AllGathers are still in flight ----
            for mt in range(MT):
                st_chain(kT16[:, :, mt * 128:(mt + 1) * 128], ("own", mt))
            u_pass([("own", mt) for mt in range(MT)],
                   lambda ck, fo2: v16[:, ck[1], fo2 * 128:(fo2 + 1) * 128])

            # ---- main phases: per half h, scores^T then U for the 7 peers
            #      (gpsimd FIFO: kt loads (AG-k h) before v loads (AG-v h)) ----
            for h in range(2):            # token-half (mt 2h, 2h+1)
                for j in range(R - 1):
                    kt_r = ktpool.tile([128, FO, SH], f16, tag="kt", name="kt_r")
                    nc.sync.dma_start(
                        kt_r[:],
                        k_gath[h][bass.ds(rot_sv[j], 1)].rearrange(
                            "o p (fo t) -> p (o fo) t", t=SH))
                    for ml in range(2):
                        mt = h * 2 + ml
                        st_chain(kt_r[:, :, ml * 128:(ml + 1) * 128], (j, mt))
                v_tiles = {}
                for j in range(R - 1):
                    v_r = vpool.tile([128, 2, F], f16, tag="v", name="v_r")
                    nc.sync.dma_start(
                        v_r[:],
                        v_gath[h][bass.ds(rot_sv[j], 1)].rearrange(
                            "o p (mt f) -> p (o mt) f", f=F))
                    v_tiles[j] = v_r
                chunks = [(j, h * 2 + ml) for j in range(R - 1)
                          for ml in range(2)]
                u_pass(chunks,
                       lambda ck, fo2, vt=v_tiles, hh=h: vt[ck[0]][
                           :, ck[1] - 2 * hh, fo2 * 128:(fo2 + 1) * 128])

            # ---- normalize: outT = U * (1/d) broadcast along partitions ----
            # d = ones^T @ t_sum  (cross-partition sum via one fp32 matmul)
            psum_d = ps_dpool.tile([1, S], f32, name="psum_d")
            nc.tensor.matmul(psum_d[:], ones_c32[:], t_sum[:],
                             start=True, stop=True)
            nc.vector.reciprocal(out=recip32[:], in_=psum_d[:])
            ps_bc = ps_one.tile([128, S], f32, tag="ps_bc", name="ps_bc")
            nc.tensor.matmul(ps_bc[:], ones_r32[:], recip32[:],
                             start=True, stop=True)
            nc.vector.tensor_copy(out=rb32[:], in_=ps_bc[:])
            for fo2 in range(FO):
                nc.vector.tensor_tensor(
                    outT16[:, fo2, :], u_sb[:, fo2, :], rb32[:],
                    mybir.AluOpType.mult)

            # ---- phase E: yT = Wout^T @ outT + bout ----
            yT_view = yT.ap().rearrange("(mo p) t -> p mo t", p=128)
            for mo in range(KO):
                ps_y = ps_proj.tile([128, S], f32, tag="ps_proj", name="ps_y")
                for fo2 in range(FO):
                    nc.tensor.matmul(
                        ps_y[:], wo16[:, fo2, mo * 128:(mo + 1) * 128],
                        outT16[:, fo2, :],
                        start=(fo2 == 0), stop=(fo2 == FO - 1))
                y_t = ypool.tile([128, S], f32, tag="y", name="y_t")
                nc.vector.tensor_scalar_add(
                    out=y_t[:], in0=ps_y[:], scalar1=bo_s[:, mo:mo + 1])
                nc.sync.dma_start(yT_view[:, mo, :], y_t[:])

    nc.compile()
    return nc


def _prep_inputs(x, w_qkv, b_qkv, w_out, b_out):
    x = np.asarray(x, dtype=np.float32)
    w_qkv = np.asarray(w_qkv, dtype=np.float32)
    b_qkv = np.asarray(b_qkv, dtype=np.float32)
    w_out = np.asarray(w_out, dtype=np.float32)
    b_out = np.asarray(b_out, dtype=np.float32)

    shared = {
        "wq": np.ascontiguousarray(w_qkv[:, :F].astype(np.float16)),
        "wk": np.ascontiguousarray(w_qkv[:, F:2 * F].astype(np.float16)),
        "wv": np.ascontiguousarray(w_qkv[:, 2 * F:].astype(np.float16)),
        "wo": np.ascontiguousarray(w_out.astype(np.float16)),
        "bq": np.ascontiguousarray(b_qkv[:F].reshape(FO, 128).T.astype(np.float32)),
        "bk": np.ascontiguousarray(
            b_qkv[F:2 * F].reshape(FO, 128).T.astype(np.float32)),
        "bvb": np.ascontiguousarray(
            np.broadcast_to(b_qkv[2 * F:], (128, F)).astype(np.float32)),
        "bo": np.ascontiguousarray(b_out.reshape(KO, 128).T.astype(np.float32)),
    }
    in_maps = []
    for i in range(R):
        m = dict(shared)
        m["xT"] = np.ascontiguousarray(
            x[i * S:(i + 1) * S, :].T.astype(np.float16))
        in_maps.append(m)
    return in_maps


def kernel(x, w_qkv, b_qkv, w_out, b_out, trace=False, **run_kwargs):
    global _cached
    if _cached is None:
        _cached = _build()
    nc = _cached
    in_maps = _prep_inputs(x, w_qkv, b_qkv, w_out, b_out)
    res = run_bass_kernel_spmd(nc, in_maps, core_ids=list(range(R)),
                               trace=trace, **run_kwargs)
    y = np.concatenate(
        [res.results[i]["yT"].T for i in range(R)], axis=0)
    kernel.last_results = res
    return np.ascontiguousarray(y, dtype=np.float32)



# revision 2
# speedup vs baseline: 1.5354x; 1.5354x over previous
"""Trainium2 Bass kernel for nn_MultiHeadAttention_64106681860559.

Fused single-score-matrix MHA: qkv = x@Wqkv+b; S = q k^T/8; attn = softmax(S);
out = (attn @ v) @ Wout + bout.   x:[4096,1024] fp32 -> y:[4096,1024] fp32.

Strategy: shard queries (dim 0) across 8 cores; ZERO collectives via weight
folding (associativity):
  scores^T = K Q^T = x (Wk Wq^T) x_own^T   with G = Wk Wq^T folded on host,
  so per core: P = G^T-chunks @ x_own^T  [1024, 512], then S^T = x @ P using
  the full (replicated) x — no K/V AllGather needed.  The key-side bias
  (x_j . Wk bq) folds into P's bias add; query-side constants cancel in
  softmax.  Attention output:
  y^T = (Wv Wo)^T (x^T E)/d + (bv Wo + bo)  with W2 = Wv Wo folded on host,
  killing the V projection and the U=V^T E evacuation: T = x^T E is a single
  accumulation over key chunks, normalized by d = ones^T E, then one output
  projection.
Per-core PE work: P (32768 cyc) + S^T (131072) + T (131072) + y^T (32768)
= 327680 cycles of fp16 matmul — ~168 us at the observed 1.95 GHz (13/16
GPIO-throttled) clock, with no collective stalls.
All matmul operands fp16, fp32 PSUM accumulation; exp on ACT with fixed
shift -16 (softmax shift-invariant); denominator via ones-vector matmul.
Measured end-to-end error vs fp32 reference (numpy sim): ~2.0e-3.
"""
import sys
import numpy as np

for _p in ("/opt/trn_rl_repo", "/root/.axon_site/_ro/trn_rl_repo"):
    if _p not in sys.path:
        sys.path.insert(0, _p)

import concourse.bass as bass  # noqa: E402
import concourse.tile as tile  # noqa: E402
from concourse import bacc, mybir  # noqa: E402
from concourse.bass_utils import run_bass_kernel_spmd  # noqa: E402

R = 8            # cores
N = 4096         # tokens
S = N // R       # 512 queries per shard
L = 1024         # latent
KO = L // 128    # 8 latent chunks
NKC = N // 128   # 32 key chunks
NQT = 4          # key-chunk quarters (8 chunks each)
EXP_SHIFT = -16.0
SCALE = 0.125    # 1/sqrt(Dk)

f16 = mybir.dt.float16
f32 = mybir.dt.float32

_cached = None


def _build():
    nc = bacc.Bacc("TRN2", target_bir_lowering=False, debug=False, num_devices=R)

    gt = nc.dram_tensor("gt", [L, L], f16, kind="ExternalInput")       # G^T
    xt = nc.dram_tensor("xt", [L, N], f16, kind="ExternalInput")       # x^T full
    xtown = nc.dram_tensor("xtown", [L, S], f16, kind="ExternalInput")  # x^T own cols
    xtok = nc.dram_tensor("xtok", [N, L], f16, kind="ExternalInput")   # x full
    w2t = nc.dram_tensor("w2t", [L, L], f16, kind="ExternalInput")     # W2 = Wv Wo
    cp = nc.dram_tensor("cp", [128, KO], f32, kind="ExternalInput")    # Wk bq
    b2 = nc.dram_tensor("b2", [128, KO], f32, kind="ExternalInput")    # bv Wo + bo
    yT = nc.dram_tensor("yT", [L, S], f32, kind="ExternalOutput")

    with tile.TileContext(nc) as tc:
        with tc.tile_pool(name="const", bufs=1) as const, \
             tc.tile_pool(name="xkpool", bufs=2) as xkpool, \
             tc.tile_pool(name="epool", bufs=16) as epool, \
             tc.tile_pool(name="ypool", bufs=2) as ypool, \
             tc.tile_pool(name="ps_a", bufs=2, space="PSUM") as ps_a, \
             tc.tile_pool(name="ps_s", bufs=2, space="PSUM") as ps_s_pool, \
             tc.tile_pool(name="ps_t", bufs=2, space="PSUM") as ps_t_pool, \
             tc.tile_pool(name="ps_d", bufs=1, space="PSUM") as ps_dpool, \
             tc.tile_pool(name="ps_one", bufs=1, space="PSUM") as ps_one:

            # ---- first-need inputs: G^T, x^T own columns, key-bias ----
            gt16 = const.tile([128, KO, L], f16, name="gt16")
            gt_view = gt.ap().rearrange("(bo p) a -> p bo a", p=128)
            # halves so P chunk 0 starts before the full load
            nc.sync.dma_start(gt16[:, :, :512], gt_view[:, :, :512])
            nc.sync.dma_start(gt16[:, :, 512:], gt_view[:, :, 512:])
            xtown16 = const.tile([128, KO, S], f16, name="xtown16")
            nc.sync.dma_start(xtown16[:], xtown.ap().rearrange("(bo p) t -> p bo t", p=128))
            cp_s = const.tile([128, KO], f32, name="cp_s")
            nc.sync.dma_start(cp_s[:], cp.ap())

            ones_c32 = const.tile([128, 1], f32, name="ones_c32")
            nc.vector.memset(ones_c32[:], 1.0)
            ones_r32 = const.tile([1, 128], f32, name="ones_r32")
            nc.vector.memset(ones_r32[:], 1.0)
            expb = const.tile([128, 1], f32, name="expb")
            nc.vector.memset(expb[:], EXP_SHIFT)

            P16 = const.tile([128, KO, S], f16, name="P16")
            xt16 = const.tile([128, KO, N], f16, name="xt16")
            t_sum = const.tile([128, S], f32, name="t_sum")
            T_sb = const.tile([128, KO, S], f32, name="T_sb")
            O16 = const.tile([128, KO, S], f16, name="O16")
            rb32 = const.tile([128, S], f32, name="rb32")
            recip32 = const.tile([1, S], f32, name="recip32")

            # ---- stream x^T (full) in column blocks for the S^T phase ----
            xt_view = xt.ap().rearrange("(lo p) t -> p lo t", p=128)
            XBLK = 1024
            for b in range(N // XBLK):
                nc.sync.dma_start(xt16[:, :, b * XBLK:(b + 1) * XBLK],
                                  xt_view[:, :, b * XBLK:(b + 1) * XBLK])

            # ---- phase P: P = G x_own^T + cvec  [1024, 512] ----
            for a in range(KO):
                ps = ps_a.tile([128, S], f32, tag="ps_a", name="ps_p")
                for bo in range(KO):
                    nc.tensor.matmul(
                        ps[:], gt16[:, bo, a * 128:(a + 1) * 128],
                        xtown16[:, bo, :],
                        start=(bo == 0), stop=(bo == KO - 1))
                nc.vector.tensor_scalar_add(
                    out=P16[:, a, :], in0=ps[:], scalar1=cp_s[:, a:a + 1])

            # deferred loads: x token-major quarters, W2^T, b2
            xtok_view = xtok.ap().rearrange("(kc p) l -> p kc l", p=128)
            xq_tiles = []
            for qt in range(2):  # first two quarters early
                xq = xkpool.tile([128, 8, L], f16, tag="xq", name=f"xq{qt}")
                nc.sync.dma_start(xq[:], xtok_view[:, qt * 8:(qt + 1) * 8, :])
                xq_tiles.append(xq)

            w2t16 = const.tile([128, KO, L], f16, name="w2t16")
            nc.sync.dma_start(w2t16[:], w2t.ap().rearrange("(fo p) m -> p fo m", p=128))
            b2_s = const.tile([128, KO], f32, name="b2_s")
            nc.sync.dma_start(b2_s[:], b2.ap())

            # ---- main loop: S^T chunks (+exp, +t_sum) and T accumulation ----
            first_e = True
            first_t = [True] * KO
            e_tiles = {}

            def st_chain(kc):
                nonlocal first_e
                ps_s = ps_s_pool.tile([128, S], f32, tag="ps_s", name="ps_s")
                for lo in range(KO):
                    nc.tensor.matmul(
                        ps_s[:], xt16[:, lo, kc * 128:(kc + 1) * 128],
                        P16[:, lo, :],
                        start=(lo == 0), stop=(lo == KO - 1))
                e_t = epool.tile([128, S], f16, tag="e", name="e_t")
                nc.scalar.activation(
                    e_t[:], ps_s[:], mybir.ActivationFunctionType.Exp,
                    bias=expb[:], scale=SCALE)
                if first_e:
                    nc.vector.tensor_copy(out=t_sum[:], in_=e_t[:])
                    first_e = False
                else:
                    nc.vector.tensor_tensor(
                        t_sum[:], t_sum[:], e_t[:], mybir.AluOpType.add)
                e_tiles[kc] = e_t

            def t_pass(qt, xq):
                for lo in range(KO):
                    ps_t = ps_t_pool.tile([128, S], f32, tag="ps_t", name="ps_t")
                    for j in range(8):
                        kc = qt * 8 + j
                        nc.tensor.matmul(
                            ps_t[:], xq[:, j, lo * 128:(lo + 1) * 128],
                            e_tiles[kc][:],
                            start=(j == 0), stop=(j == 7))
                    if first_t[lo]:
                        nc.vector.tensor_copy(out=T_sb[:, lo, :], in_=ps_t[:])
                        first_t[lo] = False
                    else:
                        nc.vector.tensor_tensor(
                            T_sb[:, lo, :], T_sb[:, lo, :], ps_t[:],
                            mybir.AluOpType.add)

            for qt in range(NQT):
                for j in range(8):
                    st_chain(qt * 8 + j)
                if qt + 2 < NQT:  # prefetch quarter qt+2
                    xq = xkpool.tile([128, 8, L], f16, tag="xq", name=f"xq{qt+2}")
                    nc.sync.dma_start(
                        xq[:], xtok_view[:, (qt + 2) * 8:(qt + 3) * 8, :])
                    xq_tiles.append(xq)
                t_pass(qt, xq_tiles[qt])

            # ---- normalize: O = T * (1/d) broadcast along partitions ----
            psum_d = ps_dpool.tile([1, S], f32, name="psum_d")
            nc.tensor.matmul(psum_d[:], ones_c32[:], t_sum[:],
                             start=True, stop=True)
            nc.vector.reciprocal(out=recip32[:], in_=psum_d[:])
            ps_bc = ps_one.tile([128, S], f32, tag="ps_bc", name="ps_bc")
            nc.tensor.matmul(ps_bc[:], ones_r32[:], recip32[:],
                             start=True, stop=True)
            nc.vector.tensor_copy(out=rb32[:], in_=ps_bc[:])
            for lo in range(KO):
                nc.vector.tensor_tensor(
                    O16[:, lo, :], T_sb[:, lo, :], rb32[:],
                    mybir.AluOpType.mult)

            # ---- output projection: yT = W2^T O + b2 ----
            yT_view = yT.ap().rearrange("(mo p) t -> p mo t", p=128)
            for mo in range(KO):
                ps_y = ps_a.tile([128, S], f32, tag="ps_a", name="ps_y")
                for fo in range(KO):
                    nc.tensor.matmul(
                        ps_y[:], w2t16[:, fo, mo * 128:(mo + 1) * 128],
                        O16[:, fo, :],
                        start=(fo == 0), stop=(fo == KO - 1))
                y_t = ypool.tile([128, S], f32, tag="y", name="y_t")
                nc.vector.tensor_scalar_add(
                    out=y_t[:], in0=ps_y[:], scalar1=b2_s[:, mo:mo + 1])
                nc.sync.dma_start(yT_view[:, mo, :], y_t[:])

    nc.compile()
    return nc


def _prep_inputs(x, w_qkv, b_qkv, w_out, b_out):
    x = np.asarray(x, dtype=np.float32)
    w_qkv = np.asarray(w_qkv, dtype=np.float32)
    b_qkv = np.asarray(b_qkv, dtype=np.float32)
    w_out = np.asarray(w_out, dtype=np.float32)
    b_out = np.asarray(b_out, dtype=np.float32)

    Wq = w_qkv[:, :L]
    Wk = w_qkv[:, L:2 * L]
    Wv = w_qkv[:, 2 * L:]
    bq = b_qkv[:L]
    bv = b_qkv[2 * L:]

    G = Wk @ Wq.T                    # [L, L]
    cvec = Wk @ bq                   # [L]
    W2 = Wv @ w_out                  # [L, L]
    b2 = bv @ w_out + b_out          # [L]

    x16 = np.ascontiguousarray(x.astype(np.float16))
    xT16 = np.ascontiguousarray(x16.T)
    shared = {
        "gt": np.ascontiguousarray(G.T.astype(np.float16)),
        "xt": xT16,
        "xtok": x16,
        "w2t": np.ascontiguousarray(W2.astype(np.float16)),
        "cp": np.ascontiguousarray(cvec.reshape(KO, 128).T.astype(np.float32)),
        "b2": np.ascontiguousarray(b2.reshape(KO, 128).T.astype(np.float32)),
    }
    in_maps = []
    for i in range(R):
        m = dict(shared)
        m["xtown"] = np.ascontiguousarray(xT16[:, i * S:(i + 1) * S])
        in_maps.append(m)
    return in_maps


def kernel(x, w_qkv, b_qkv, w_out, b_out, trace=False, **run_kwargs):
    global _cached
    if _cached is None:
        _cached = _build()
    nc = _cached
    in_maps = _prep_inputs(x, w_qkv, b_qkv, w_out, b_out)
    res = run_bass_kernel_spmd(nc, in_maps, core_ids=list(range(R)),
                               trace=trace, **run_kwargs)
    y = np.concatenate(
        [res.results[i]["yT"].T for i in range(R)], axis=0)
    kernel.last_results = res
    return np.ascontiguousarray(y, dtype=np.float32)


# revision 3
# speedup vs baseline: 1.5388x; 1.0022x over previous
"""Trainium2 Bass kernel for nn_MultiHeadAttention_64106681860559.

Fused single-score-matrix MHA: qkv = x@Wqkv+b; S = q k^T/8; attn = softmax(S);
out = (attn @ v) @ Wout + bout.   x:[4096,1024] fp32 -> y:[4096,1024] fp32.

Strategy: shard queries (dim 0) across 8 cores; ZERO collectives via weight
folding (associativity):
  scores^T = K Q^T = x (Wk Wq^T) x_own^T   with G = Wk Wq^T folded on host,
  so per core: P = G^T-chunks @ x_own^T  [1024, 512], then S^T = x @ P using
  the full (replicated) x — no K/V AllGather needed.  The key-side bias
  (x_j . Wk bq) folds into P's bias add; query-side constants cancel in
  softmax.  Attention output:
  y^T = (Wv Wo)^T (x^T E) * (1/d) + (bv Wo + bo)  with W2 = Wv Wo folded on
  host; T = x^T E is accumulated unnormalized (absmax ~2.6e4, fp16-safe) and
  the per-query 1/d scale commutes with the projection, so it is applied in
  the final evacuation — the denominator chain overlaps the projection.
Per-core PE work: P (32768 cyc) + S^T (131072) + T (131072) + y^T (32768)
= 327680 cycles of fp16 matmul (~137 us at 2.4 GHz).
All matmul operands fp16, fp32 PSUM accumulation; exp on ACT with fixed
shift -16 (softmax shift-invariant); denominator via ones-vector matmul.
Measured end-to-end error vs fp32 reference (numpy sim): ~2.0e-3.
"""
import sys
import numpy as np

for _p in ("/opt/trn_rl_repo", "/root/.axon_site/_ro/trn_rl_repo"):
    if _p not in sys.path:
        sys.path.insert(0, _p)

import concourse.bass as bass  # noqa: E402
import concourse.tile as tile  # noqa: E402
from concourse import bacc, mybir  # noqa: E402
from concourse.bass_utils import run_bass_kernel_spmd  # noqa: E402

R = 8            # cores
N = 4096         # tokens
S = N // R       # 512 queries per shard
L = 1024         # latent
KO = L // 128    # 8 latent chunks
NKC = N // 128   # 32 key chunks
NQT = 4          # key-chunk quarters (8 chunks each)
EXP_SHIFT = -16.0
SCALE = 0.125    # 1/sqrt(Dk)

f16 = mybir.dt.float16
f32 = mybir.dt.float32

_cached = None


def _build():
    nc = bacc.Bacc("TRN2", target_bir_lowering=False, debug=False, num_devices=R)

    gt = nc.dram_tensor("gt", [L, L], f16, kind="ExternalInput")       # G^T
    xt = nc.dram_tensor("xt", [L, N], f16, kind="ExternalInput")       # x^T full
    xtown = nc.dram_tensor("xtown", [L, S], f16, kind="ExternalInput")  # x^T own cols
    xtok = nc.dram_tensor("xtok", [N, L], f16, kind="ExternalInput")   # x full
    w2t = nc.dram_tensor("w2t", [L, L], f16, kind="ExternalInput")     # W2 = Wv Wo
    cp = nc.dram_tensor("cp", [128, KO], f32, kind="ExternalInput")    # Wk bq
    b2 = nc.dram_tensor("b2", [128, KO], f32, kind="ExternalInput")    # bv Wo + bo
    yT = nc.dram_tensor("yT", [L, S], f32, kind="ExternalOutput")

    with tile.TileContext(nc) as tc:
        with tc.tile_pool(name="const", bufs=1) as const, \
             tc.tile_pool(name="xkpool", bufs=2) as xkpool, \
             tc.tile_pool(name="epool", bufs=16) as epool, \
             tc.tile_pool(name="ypool", bufs=2) as ypool, \
             tc.tile_pool(name="ps_a", bufs=1, space="PSUM") as ps_a, \
             tc.tile_pool(name="ps_s", bufs=4, space="PSUM") as ps_s_pool, \
             tc.tile_pool(name="ps_t", bufs=2, space="PSUM") as ps_t_pool, \
             tc.tile_pool(name="ps_d", bufs=1, space="PSUM") as ps_dpool:

            # ---- first-need inputs: G^T half 1, x^T own columns, key-bias ----
            gt16 = const.tile([128, KO, L], f16, name="gt16")
            gt_view = gt.ap().rearrange("(bo p) a -> p bo a", p=128)
            nc.sync.dma_start(gt16[:, :, :512], gt_view[:, :, :512])
            xtown16 = const.tile([128, KO, S], f16, name="xtown16")
            nc.sync.dma_start(xtown16[:], xtown.ap().rearrange("(bo p) t -> p bo t", p=128))
            cp_s = const.tile([128, KO], f32, name="cp_s")
            nc.sync.dma_start(cp_s[:], cp.ap())
            nc.sync.dma_start(gt16[:, :, 512:], gt_view[:, :, 512:])

            ones_c32 = const.tile([128, 1], f32, name="ones_c32")
            nc.vector.memset(ones_c32[:], 1.0)
            ones_r32 = const.tile([1, 128], f32, name="ones_r32")
            nc.vector.memset(ones_r32[:], 1.0)
            expb = const.tile([128, 1], f32, name="expb")
            nc.vector.memset(expb[:], EXP_SHIFT)

            P16 = const.tile([128, KO, S], f16, name="P16")
            xt16 = const.tile([128, KO, N], f16, name="xt16")
            t_sum = const.tile([128, S], f32, name="t_sum")
            T_sb = const.tile([128, KO, S], f32, name="T_sb")
            T16 = const.tile([128, KO, S], f16, name="T16")
            rb32 = const.tile([128, S], f32, name="rb32")
            recip32 = const.tile([1, S], f32, name="recip32")

            # ---- phase P: P = G x_own^T + cvec  [1024, 512] ----
            def p_chunk(a):
                ps = ps_a.tile([128, S], f32, tag="ps_a", name="ps_p")
                for bo in range(KO):
                    nc.tensor.matmul(
                        ps[:], gt16[:, bo, a * 128:(a + 1) * 128],
                        xtown16[:, bo, :],
                        start=(bo == 0), stop=(bo == KO - 1))
                nc.vector.tensor_scalar_add(
                    out=P16[:, a, :], in0=ps[:], scalar1=cp_s[:, a:a + 1])

            for a in range(4):
                p_chunk(a)          # needs only gt half 1

            # stream x^T (full) in column blocks for the S^T phase
            xt_view = xt.ap().rearrange("(lo p) t -> p lo t", p=128)
            XBLK = 1024
            for b in range(N // XBLK):
                nc.sync.dma_start(xt16[:, :, b * XBLK:(b + 1) * XBLK],
                                  xt_view[:, :, b * XBLK:(b + 1) * XBLK])

            for a in range(4, KO):
                p_chunk(a)          # needs gt half 2

            # deferred loads: x token-major quarters, W2^T, b2
            xtok_view = xtok.ap().rearrange("(kc p) l -> p kc l", p=128)
            xq_tiles = []
            for qt in range(2):
                xq = xkpool.tile([128, 8, L], f16, tag="xq", name=f"xq{qt}")
                nc.sync.dma_start(xq[:], xtok_view[:, qt * 8:(qt + 1) * 8, :])
                xq_tiles.append(xq)

            w2t16 = const.tile([128, KO, L], f16, name="w2t16")
            nc.sync.dma_start(w2t16[:], w2t.ap().rearrange("(fo p) m -> p fo m", p=128))
            b2_s = const.tile([128, KO], f32, name="b2_s")
            nc.sync.dma_start(b2_s[:], b2.ap())

            # ---- main loop: S^T chunks (+exp, +t_sum) and T accumulation ----
            first_e = True
            e_tiles = {}

            def st_chain(kc):
                nonlocal first_e
                ps_s = ps_s_pool.tile([128, S], f32, tag="ps_s", name="ps_s")
                for lo in range(KO):
                    nc.tensor.matmul(
                        ps_s[:], xt16[:, lo, kc * 128:(kc + 1) * 128],
                        P16[:, lo, :],
                        start=(lo == 0), stop=(lo == KO - 1))
                e_t = epool.tile([128, S], f16, tag="e", name="e_t")
                nc.scalar.activation(
                    e_t[:], ps_s[:], mybir.ActivationFunctionType.Exp,
                    bias=expb[:], scale=SCALE)
                if first_e:
                    nc.vector.tensor_copy(out=t_sum[:], in_=e_t[:])
                    first_e = False
                else:
                    nc.vector.tensor_tensor(
                        t_sum[:], t_sum[:], e_t[:], mybir.AluOpType.add)
                e_tiles[kc] = e_t

            def t_pass(qt, xq):
                last = (qt == NQT - 1)
                for lo in range(KO):
                    ps_t = ps_t_pool.tile([128, S], f32, tag="ps_t", name="ps_t")
                    for j in range(8):
                        kc = qt * 8 + j
                        nc.tensor.matmul(
                            ps_t[:], xq[:, j, lo * 128:(lo + 1) * 128],
                            e_tiles[kc][:],
                            start=(j == 0), stop=(j == 7))
                    if qt == 0:
                        nc.vector.tensor_copy(out=T_sb[:, lo, :], in_=ps_t[:])
                    elif not last:
                        nc.vector.tensor_tensor(
                            T_sb[:, lo, :], T_sb[:, lo, :], ps_t[:],
                            mybir.AluOpType.add)
                    else:   # final quarter: fused add + fp16 convert
                        nc.vector.tensor_tensor(
                            T16[:, lo, :], T_sb[:, lo, :], ps_t[:],
                            mybir.AluOpType.add)

            for qt in range(NQT):
                for j in range(8):
                    st_chain(qt * 8 + j)
                if qt + 2 < NQT:
                    xq = xkpool.tile([128, 8, L], f16, tag="xq", name=f"xq{qt+2}")
                    nc.sync.dma_start(
                        xq[:], xtok_view[:, (qt + 2) * 8:(qt + 3) * 8, :])
                    xq_tiles.append(xq)
                t_pass(qt, xq_tiles[qt])

            # ---- denominator chain (overlaps output projection) ----
            psum_d = ps_dpool.tile([1, S], f32, tag="ps_d", name="psum_d")
            nc.tensor.matmul(psum_d[:], ones_c32[:], t_sum[:],
                             start=True, stop=True)
            nc.vector.reciprocal(out=recip32[:], in_=psum_d[:])
            ps_bc = ps_dpool.tile([128, S], f32, tag="ps_d", name="ps_bc")
            nc.tensor.matmul(ps_bc[:], ones_r32[:], recip32[:],
                             start=True, stop=True)
            nc.vector.tensor_copy(out=rb32[:], in_=ps_bc[:])

            # ---- output projection: yT = (W2^T T) * rb + b2 ----
            yT_view = yT.ap().rearrange("(mo p) t -> p mo t", p=128)
            for mo in range(KO):
                ps_y = ps_a.tile([128, S], f32, tag="ps_a", name="ps_y")
                for fo in range(KO):
                    nc.tensor.matmul(
                        ps_y[:], w2t16[:, fo, mo * 128:(mo + 1) * 128],
                        T16[:, fo, :],
                        start=(fo == 0), stop=(fo == KO - 1))
                y_t = ypool.tile([128, S], f32, tag="y", name="y_t")
                nc.vector.tensor_tensor(
                    y_t[:], ps_y[:], rb32[:], mybir.AluOpType.mult)
                nc.vector.tensor_scalar_add(
                    out=y_t[:], in0=y_t[:], scalar1=b2_s[:, mo:mo + 1])
                nc.sync.dma_start(yT_view[:, mo, :], y_t[:])

    nc.compile()
    return nc


def _prep_inputs(x, w_qkv, b_qkv, w_out, b_out):
    x = np.asarray(x, dtype=np.float32)
    w_qkv = np.asarray(w_qkv, dtype=np.float32)
    b_qkv = np.asarray(b_qkv, dtype=np.float32)
    w_out = np.asarray(w_out, dtype=np.float32)
    b_out = np.asarray(b_out, dtype=np.float32)

    Wq = w_qkv[:, :L]
    Wk = w_qkv[:, L:2 * L]
    Wv = w_qkv[:, 2 * L:]
    bq = b_qkv[:L]
    bv = b_qkv[2 * L:]

    G = Wk @ Wq.T                    # [L, L]
    cvec = Wk @ bq                   # [L]
    W2 = Wv @ w_out                  # [L, L]
    b2 = bv @ w_out + b_out          # [L]

    x16 = np.ascontiguousarray(x.astype(np.float16))
    xT16 = np.ascontiguousarray(x16.T)
    shared = {
        "gt": np.ascontiguousarray(G.T.astype(np.float16)),
        "xt": xT16,
        "xtok": x16,
        "w2t": np.ascontiguousarray(W2.astype(np.float16)),
        "cp": np.ascontiguousarray(cvec.reshape(KO, 128).T.astype(np.float32)),
        "b2": np.ascontiguousarray(b2.reshape(KO, 128).T.astype(np.float32)),
    }
    in_maps = []
    for i in range(R):
        m = dict(shared)
        m["xtown"] = np.ascontiguousarray(xT16[:, i * S:(i + 1) * S])
        in_maps.append(m)
    return in_maps


def kernel(x, w_qkv, b_qkv, w_out, b_out, trace=False, **run_kwargs):
    global _cached
    if _cached is None:
        _cached = _build()
    nc = _cached
    in_maps = _prep_inputs(x, w_qkv, b_qkv, w_out, b_out)
    res = run_bass_kernel_spmd(nc, in_maps, core_ids=list(range(R)),
                               trace=trace, **run_kwargs)
    y = np.concatenate(
        [res.results[i]["yT"].T for i in range(R)], axis=0)
    kernel.last_results = res
    return np.ascontiguousarray(y, dtype=np.float32)


# revision 7
# speedup vs baseline: 1.6204x; 1.0530x over previous
"""Trainium2 Bass kernel for nn_MultiHeadAttention_64106681860559.

Fused single-score-matrix MHA: qkv = x@Wqkv+b; S = q k^T/8; attn = softmax(S);
out = (attn @ v) @ Wout + bout.   x:[4096,1024] fp32 -> y:[4096,1024] fp32.

Strategy: shard queries (dim 0) across 8 cores; ZERO collectives via weight
folding (associativity):
  scores^T = K Q^T = x (Wk Wq^T) x_own^T   with G = Wk Wq^T folded on host,
  so per core: P = G^T-chunks @ x_own^T  [1024, 512], then S^T = x @ P using
  the full (replicated) x — no K/V AllGather needed.  The key-side bias
  (x_j . Wk bq) folds into P's bias add; query-side constants cancel in
  softmax.  Attention output:
  y^T = (Wv Wo)^T (x^T E) * (1/d) + (bv Wo + bo)  with W2 = Wv Wo folded on
  host; T = x^T E is accumulated unnormalized (absmax ~2.6e4, fp16-safe) and
  the per-query 1/d scale commutes with the projection, so it is applied in
  the final evacuation — the denominator chain overlaps the projection.
Per-core PE work: P (32768 cyc) + S^T (131072) + T (131072) + y^T (32768)
= 327680 cycles of fp16 matmul (~137 us at 2.4 GHz).
All matmul operands fp16, fp32 PSUM accumulation; exp on ACT with fixed
shift -16 (softmax shift-invariant); denominator via ones-vector matmul.
Measured end-to-end error vs fp32 reference (numpy sim): ~2.0e-3.
"""
import sys
import numpy as np

for _p in ("/opt/trn_rl_repo", "/root/.axon_site/_ro/trn_rl_repo"):
    if _p not in sys.path:
        sys.path.insert(0, _p)

import concourse.bass as bass  # noqa: E402
import concourse.tile as tile  # noqa: E402
from concourse import bacc, mybir  # noqa: E402
from concourse.bass_utils import run_bass_kernel_spmd  # noqa: E402

R = 8            # cores
N = 4096         # tokens
S = N // R       # 512 queries per shard
L = 1024         # latent
KO = L // 128    # 8 latent chunks
NKC = N // 128   # 32 key chunks
NQT = 4          # key-chunk quarters (8 chunks each)
EXP_SHIFT = -16.0
SCALE = 0.125    # 1/sqrt(Dk)

f16 = mybir.dt.float16
f32 = mybir.dt.float32

_cached = None


def _build():
    nc = bacc.Bacc("TRN2", target_bir_lowering=False, debug=False, num_devices=R)

    gt = nc.dram_tensor("gt", [L, L], f16, kind="ExternalInput")       # G^T
    xt = nc.dram_tensor("xt", [L, N], f16, kind="ExternalInput")       # x^T full
    xtown = nc.dram_tensor("xtown", [L, S], f16, kind="ExternalInput")  # x^T own cols
    xtok = nc.dram_tensor("xtok", [N, L], f16, kind="ExternalInput")   # x full
    w2t = nc.dram_tensor("w2t", [L, L], f16, kind="ExternalInput")     # W2 = Wv Wo
    cp = nc.dram_tensor("cp", [128, KO], f32, kind="ExternalInput")    # Wk bq
    b2 = nc.dram_tensor("b2", [128, KO], f32, kind="ExternalInput")    # bv Wo + bo
    yT = nc.dram_tensor("yT", [L, S], f32, kind="ExternalOutput")

    with tile.TileContext(nc) as tc:
        with tc.tile_pool(name="const", bufs=1) as const, \
             tc.tile_pool(name="xkpool", bufs=2) as xkpool, \
             tc.tile_pool(name="epool", bufs=16) as epool, \
             tc.tile_pool(name="ypool", bufs=2) as ypool, \
             tc.tile_pool(name="ps_a", bufs=2, space="PSUM") as ps_a, \
             tc.tile_pool(name="ps_s", bufs=4, space="PSUM") as ps_s_pool, \
             tc.tile_pool(name="ps_t", bufs=2, space="PSUM") as ps_t_pool:

            # ---- PE warmup: dummy matmuls during the initial DMA wait keep
            #      the HAM activity window busy so P runs at full clock ----
            warm16 = const.tile([128, 64], f16, name="warm16")
            nc.vector.memset(warm16[:], 0.0)
            ps_w = ps_a.tile([128, 64], f32, tag="ps_a", name="ps_w")
            for _ in range(56):
                nc.tensor.matmul(ps_w[:64, :], warm16[:, :64],
                                 warm16[:, :64], start=True, stop=True)

            # ---- first-need inputs: G^T (a-slices), x^T own cols, key-bias ----
            gt16 = const.tile([128, KO, L], f16, name="gt16")
            gt_view = gt.ap().rearrange("(bo p) a -> p bo a", p=128)
            xtown16 = const.tile([128, KO, S], f16, name="xtown16")
            xtown_view = xtown.ap().rearrange("(bo p) t -> p bo t", p=128)
            nc.sync.dma_start(gt16[:, :, :128], gt_view[:, :, :128])
            nc.sync.dma_start(xtown16[:, :4, :], xtown_view[:, :4, :])
            nc.sync.dma_start(gt16[:, :, 128:256], gt_view[:, :, 128:256])
            nc.sync.dma_start(xtown16[:, 4:, :], xtown_view[:, 4:, :])
            cp_s = const.tile([128, KO], f32, name="cp_s")
            nc.sync.dma_start(cp_s[:], cp.ap())
            for a in range(2, KO):
                nc.sync.dma_start(gt16[:, :, a * 128:(a + 1) * 128],
                                  gt_view[:, :, a * 128:(a + 1) * 128])

            ones_c32 = const.tile([128, 1], f32, name="ones_c32")
            nc.vector.memset(ones_c32[:], 1.0)
            ones_r32 = const.tile([1, 128], f32, name="ones_r32")
            nc.vector.memset(ones_r32[:], 1.0)
            expb = const.tile([128, 1], f32, name="expb")
            nc.vector.memset(expb[:], EXP_SHIFT)

            P16 = const.tile([128, KO, S], f16, name="P16")
            xt16 = const.tile([128, KO, N], f16, name="xt16")
            t_sum = const.tile([128, S], f32, name="t_sum")
            T_sb = const.tile([128, KO, S], f32, name="T_sb")
            T16 = const.tile([128, KO, S], f16, name="T16")
            rb32 = const.tile([128, S], f32, name="rb32")
            recip32 = const.tile([1, S], f32, name="recip32")

            # ---- phase P: P = G x_own^T + cvec  [1024, 512] ----
            def p_chunk(a):
                ps = ps_a.tile([128, S], f32, tag="ps_a", name="ps_p")
                for bo in range(KO):
                    nc.tensor.matmul(
                        ps[:], gt16[:, bo, a * 128:(a + 1) * 128],
                        xtown16[:, bo, :],
                        start=(bo == 0), stop=(bo == KO - 1))
                nc.vector.tensor_scalar_add(
                    out=P16[:, a, :], in0=ps[:], scalar1=cp_s[:, a:a + 1])

            for a in range(KO):
                p_chunk(a)

            # stream x^T (full) / x (token-major) in need order on the queue
            xt_view = xt.ap().rearrange("(lo p) t -> p lo t", p=128)
            xtok_view = xtok.ap().rearrange("(kc p) l -> p kc l", p=128)
            XBLK = 1024
            xq_tiles = []

            def load_xq(qt):
                xq = xkpool.tile([128, 8, L], f16, tag="xq", name=f"xq{qt}")
                nc.sync.dma_start(xq[:], xtok_view[:, qt * 8:(qt + 1) * 8, :])
                xq_tiles.append(xq)

            for b in range(2):
                nc.sync.dma_start(xt16[:, :, b * XBLK:(b + 1) * XBLK],
                                  xt_view[:, :, b * XBLK:(b + 1) * XBLK])
            load_xq(0)
            nc.sync.dma_start(xt16[:, :, 2 * XBLK:3 * XBLK],
                              xt_view[:, :, 2 * XBLK:3 * XBLK])
            load_xq(1)
            nc.sync.dma_start(xt16[:, :, 3 * XBLK:],
                              xt_view[:, :, 3 * XBLK:])

            w2t16 = const.tile([128, KO, L], f16, name="w2t16")
            nc.sync.dma_start(w2t16[:], w2t.ap().rearrange("(fo p) m -> p fo m", p=128))
            b2_s = const.tile([128, KO], f32, name="b2_s")
            nc.sync.dma_start(b2_s[:], b2.ap())

            # ---- main loop: S^T chunks (+exp, +t_sum) and T accumulation ----
            first_e = True
            e_tiles = {}

            def st_chain(kc):
                nonlocal first_e
                ps_s = ps_s_pool.tile([128, S], f32, tag="ps_s", name="ps_s")
                for lo in range(KO):
                    nc.tensor.matmul(
                        ps_s[:], xt16[:, lo, kc * 128:(kc + 1) * 128],
                        P16[:, lo, :],
                        start=(lo == 0), stop=(lo == KO - 1))
                e_t = epool.tile([128, S], f16, tag="e", name="e_t")
                nc.scalar.activation(
                    e_t[:], ps_s[:], mybir.ActivationFunctionType.Exp,
                    bias=expb[:], scale=SCALE)
                if first_e:
                    nc.vector.tensor_copy(out=t_sum[:], in_=e_t[:])
                    first_e = False
                else:
                    nc.vector.tensor_tensor(
                        t_sum[:], t_sum[:], e_t[:], mybir.AluOpType.add)
                e_tiles[kc] = e_t

            def t_pass(qt, xq):
                last = (qt == NQT - 1)
                for lo in range(KO):
                    ps_t = ps_t_pool.tile([128, S], f32, tag="ps_t", name="ps_t")
                    for j in range(8):
                        kc = qt * 8 + j
                        nc.tensor.matmul(
                            ps_t[:], xq[:, j, lo * 128:(lo + 1) * 128],
                            e_tiles[kc][:],
                            start=(j == 0), stop=(j == 7))
                    if qt == 0:
                        nc.vector.tensor_copy(out=T_sb[:, lo, :], in_=ps_t[:])
                    elif not last:
                        nc.vector.tensor_tensor(
                            T_sb[:, lo, :], T_sb[:, lo, :], ps_t[:],
                            mybir.AluOpType.add)
                    else:   # final quarter: fused add + fp16 convert
                        nc.vector.tensor_tensor(
                            T16[:, lo, :], T_sb[:, lo, :], ps_t[:],
                            mybir.AluOpType.add)

            for qt in range(NQT):
                for j in range(8):
                    st_chain(qt * 8 + j)
                if qt + 2 < NQT:
                    load_xq(qt + 2)
                t_pass(qt, xq_tiles[qt])

            def _evac_y(ps_y, mo, yT_view):
                y1 = ypool.tile([128, S], f32, tag="y1", name="y1")
                nc.vector.tensor_tensor(
                    y1[:], ps_y[:], rb32[:], mybir.AluOpType.mult)
                y_t = ypool.tile([128, S], f32, tag="y", name="y_t")
                nc.scalar.activation(
                    y_t[:], y1[:], mybir.ActivationFunctionType.Identity,
                    bias=b2_s[:, mo:mo + 1], scale=1.0)
                nc.sync.dma_start(yT_view[:, mo, :], y_t[:])

            # ---- output projection: yT = (W2^T T) * rb + b2; the
            #      denominator chain is sandwiched between the first chunks
            #      so its t_sum wait overlaps projection matmuls ----
            yT_view = yT.ap().rearrange("(mo p) t -> p mo t", p=128)
            ps_ys = {}
            for mo in range(KO):
                ps_y = ps_a.tile([128, S], f32, tag="ps_a", name="ps_y")
                for fo in range(KO):
                    nc.tensor.matmul(
                        ps_y[:], w2t16[:, fo, mo * 128:(mo + 1) * 128],
                        T16[:, fo, :],
                        start=(fo == 0), stop=(fo == KO - 1))
                ps_ys[mo] = ps_y
                if mo == 0:
                    psum_d = ps_t_pool.tile([1, S], f32, tag="ps_t",
                                            name="psum_d")
                    nc.tensor.matmul(psum_d[:], ones_c32[:], t_sum[:],
                                     start=True, stop=True)
                    nc.vector.reciprocal(out=recip32[:], in_=psum_d[:])
                elif mo == 1:
                    ps_bc = ps_t_pool.tile([128, S], f32, tag="ps_t",
                                           name="ps_bc")
                    nc.tensor.matmul(ps_bc[:], ones_r32[:], recip32[:],
                                     start=True, stop=True)
                    nc.vector.tensor_copy(out=rb32[:], in_=ps_bc[:])
                if mo >= 1:
                    _evac_y(ps_ys.pop(mo - 1), mo - 1, yT_view)
            _evac_y(ps_ys.pop(KO - 1), KO - 1, yT_view)

    nc.compile()
    return nc


def _prep_inputs(x, w_qkv, b_qkv, w_out, b_out):
    x = np.asarray(x, dtype=np.float32)
    w_qkv = np.asarray(w_qkv, dtype=np.float32)
    b_qkv = np.asarray(b_qkv, dtype=np.float32)
    w_out = np.asarray(w_out, dtype=np.float32)
    b_out = np.asarray(b_out, dtype=np.float32)

    Wq = w_qkv[:, :L]
    Wk = w_qkv[:, L:2 * L]
    Wv = w_qkv[:, 2 * L:]
    bq = b_qkv[:L]
    bv = b_qkv[2 * L:]

    G = Wk @ Wq.T                    # [L, L]
    cvec = Wk @ bq                   # [L]
    W2 = Wv @ w_out                  # [L, L]
    b2 = bv @ w_out + b_out          # [L]

    x16 = np.ascontiguousarray(x.astype(np.float16))
    xT16 = np.ascontiguousarray(x16.T)
    shared = {
        "gt": np.ascontiguousarray(G.T.astype(np.float16)),
        "xt": xT16,
        "xtok": x16,
        "w2t": np.ascontiguousarray(W2.astype(np.float16)),
        "cp": np.ascontiguousarray(cvec.reshape(KO, 128).T.astype(np.float32)),
        "b2": np.ascontiguousarray(b2.reshape(KO, 128).T.astype(np.float32)),
    }
    in_maps = []
    for i in range(R):
        m = dict(shared)
        m["xtown"] = np.ascontiguousarray(xT16[:, i * S:(i + 1) * S])
        in_maps.append(m)
    return in_maps


def kernel(x, w_qkv, b_qkv, w_out, b_out, trace=False, **run_kwargs):
    global _cached
    if _cached is None:
        _cached = _build()
    nc = _cached
    in_maps = _prep_inputs(x, w_qkv, b_qkv, w_out, b_out)
    res = run_bass_kernel_spmd(nc, in_maps, core_ids=list(range(R)),
                               trace=trace, **run_kwargs)
    y = np.concatenate(
        [res.results[i]["yT"].T for i in range(R)], axis=0)
    kernel.last_results = res
    return np.ascontiguousarray(y, dtype=np.float32)
